# revision 34
# baseline (speedup 1.0000x reference)
"""Distributed causal self-attention for 8 TRN2 NeuronCores (v2).

Sharding: tensor-parallel over heads (2 heads/core, all batches); an
AllToAll (split in 2 phases) redistributes the attention output from
head-sharded to token-sharded for the output projection.

v2 structure (per core, all matmuls bf16, fp32 PSUM):
  - qw-major rounds interleaving QKV projection (stage A) with attention
    (stage B): round r computes scores/exp/PV for query window r of all
    batches while the PE also runs stage-A matmuls for window r+1, so the
    ACT engine's exp stream hides under PE work.
  - scores for the two heads are emitted back-to-back as 64-row PE tiles
    (lhsT/rhs base partitions 0 and 64) -> they run concurrently in the
    PE array (row tiling).
  - per key-chunk PSUM tile [128, 1024]: cols 0:512 head0, 512:1024
    head1; ONE exp instruction per chunk with a rank-3 AP that skips the
    causally-dead prefix of diagonal chunks.  Causal triangle masked by a
    bf16 0/1 multiply on the [128,128] diagonal tiles only.
  - PV accumulates O^T (rows 0:64) and the softmax denominator (row 64,
    via a ones column baked into v_sb) in PSUM; the UNNORMALIZED output
    plus denominator rows travel through the AllToAll ([8, 130, 512]
    payload) and normalization happens on the receive side: one fast
    reciprocal + gpsimd partition-broadcasts + one big DVE multiply per
    phase.
"""

import numpy as np

import concourse.bass as bass
import concourse.bacc as bacc
import concourse.mybir as mybir
import concourse.tile as tile
from concourse.bass_utils import run_bass_kernel_spmd
from concourse.dve_ops import RECIPROCAL_APPROX_FAST, RECIP_APPROX_FAST_CONSTS

B, T, C = 4, 2048, 1024
H, D = 16, 64
NCORES = 8
HPC = H // NCORES        # heads per core
DH = HPC * D             # 128 qkv cols per core
P = 128
F32 = mybir.dt.float32
BF16 = mybir.dt.bfloat16
SCALE = 1.0 / np.sqrt(D)


def build_nc(Tb=T, reps=1, debug=False, do_norm=True):
    BT = B * Tb              # total tokens
    NTW = BT // 512          # 512-token windows (stage A units)
    NQW = Tb // 512          # query windows per batch
    NCH = BT // 128          # 128-token chunks total
    TOKS = BT // NCORES      # tokens per core in the proj stage
    NNW = C // 512           # output column windows
    NPH = 2 if Tb >= 2048 else 1
    HTOK = TOKS // NPH       # tokens per proj phase

    nc = bacc.Bacc(None, target_bir_lowering=False)

    xT_ext = nc.declare_dram_parameter("xT", [C, BT], BF16, isOutput=False)
    wq_ext = nc.declare_dram_parameter("wq", [C, DH], BF16, isOutput=False)
    wk_ext = nc.declare_dram_parameter("wk", [C, DH], BF16, isOutput=False)
    wv_ext = nc.declare_dram_parameter("wv", [C, DH], BF16, isOutput=False)
    wp_ext = nc.declare_dram_parameter("wproj", [C, C], BF16, isOutput=False)
    mk_ext = nc.declare_dram_parameter("masks", [P, 256], BF16, isOutput=False)
    sel_ext = nc.declare_dram_parameter("sel", [8, 16, P], BF16, isOutput=False)
    id_ext = nc.declare_dram_parameter("ident", [P, P], BF16, isOutput=False)
    y_ext = nc.declare_dram_parameter("y", [TOKS, C], F32, isOutput=True)
    dbg_ext = (nc.declare_dram_parameter("dbg", [NPH, 2, 16, HTOK], F32,
                                         isOutput=True) if debug else None)
    dbg2_ext = (nc.declare_dram_parameter("dbg2", [3, P, 1024], F32,
                                          isOutput=True) if debug else None)
    dbg3_ext = (nc.declare_dram_parameter("dbg3", [2, 130, 512], BF16,
                                          isOutput=True) if debug else None)
    dbg4_ext = (nc.declare_dram_parameter("dbg4", [P, 8, HTOK], BF16,
                                          isOutput=True) if debug else None)

    xT_v = xT_ext.rearrange("(c p) t -> p c t", p=P)     # [128, 8, BT]
    wq_v = wq_ext.rearrange("(c p) m -> p c m", p=P)
    wk_v = wk_ext.rearrange("(c p) m -> p c m", p=P)
    wv_v = wv_ext.rearrange("(c p) m -> p c m", p=P)
    wp_v = wp_ext.rearrange("(c p) m -> p c m", p=P)     # [128, 8, 1024]

    with tile.TileContext(nc, num_cores=NCORES) as tc:
        with (
            tc.tile_pool(name="consts", bufs=1) as consts,
            tc.tile_pool(name="acts", bufs=1) as acts,
            tc.tile_pool(name="xin", bufs=2) as xin,
            tc.tile_pool(name="small", bufs=4) as small,
            tc.tile_pool(name="ptiles", bufs=6) as ptiles,
            tc.tile_pool(name="psum", bufs=1, space="PSUM") as psum,
            tc.tile_pool(name="dram", bufs=1, space="DRAM") as dram,
        ):
            # ---- constants ----
            wq_sb = consts.tile([P, 8, DH], BF16)
            wk_sb = consts.tile([P, 8, DH], BF16)
            wv_sb = consts.tile([P, 8, DH], BF16)
            wp_sb = consts.tile([P, 8, C], BF16)
            mk_sb = consts.tile([P, 256], BF16)   # [tri | tri]
            id_sb = consts.tile([P, P], BF16)
            sel_sb = consts.tile([8, 16, P], BF16)
            nc.gpsimd.dma_start(sel_sb[:], sel_ext[:])
            nc.gpsimd.dma_start(wq_sb[:], wq_v[:])
            nc.gpsimd.dma_start(wk_sb[:], wk_v[:])
            nc.gpsimd.dma_start(wv_sb[:], wv_v[:])
            nc.gpsimd.dma_start(wp_sb[:], wp_v[:])
            nc.gpsimd.dma_start(mk_sb[:], mk_ext[:])
            nc.gpsimd.dma_start(id_sb[:], id_ext[:])

            # ---- persistent activations ----
            qT_sb = acts.tile([P, BT], BF16)
            kT_sb = acts.tile([P, BT], BF16)
            v_sb = acts.tile([P, 130 * NCH], BF16)
            nc.vector.memset(v_sb[:], 1.0)  # bakes in the ones columns

            a2a_in = [dram.tile([NCORES, 130, HTOK], BF16, name=f"a2ain{p}",
                                tag=f"a2ain{p}") for p in range(NPH)]
            a2a_out = [dram.tile([NCORES, 130, HTOK], BF16, name=f"a2aout{p}",
                                 tag=f"a2aout{p}") for p in range(NPH)]

            def stage_a_units(tw):
                """QKV projection for one 512-token window, split into 4
                filler units (~1-2us of PE work each)."""
                st = {}

                def u1():
                    xw = xin.tile([P, 8, 512], BF16, tag="xw")
                    nc.sync.dma_start(xw[:], xT_v[:, :, 512 * tw: 512 * (tw + 1)])
                    st["xw"] = xw
                    pq = psum.tile([P, 512], F32, tag="stA", bufs=2)
                    for cc in range(8):
                        nc.tensor.matmul(pq[:], wq_sb[:, cc, :], xw[:, cc, :],
                                         start=(cc == 0), stop=(cc == 7))
                    nc.vector.tensor_copy(qT_sb[:, 512 * tw: 512 * (tw + 1)], pq[:])

                def u2():
                    xw = st["xw"]
                    pk = psum.tile([P, 512], F32, tag="stA", bufs=2)
                    for cc in range(8):
                        nc.tensor.matmul(pk[:], wk_sb[:, cc, :], xw[:, cc, :],
                                         start=(cc == 0), stop=(cc == 7))
                    nc.vector.tensor_copy(kT_sb[:, 512 * tw: 512 * (tw + 1)], pk[:])

                def u3():
                    xw = st["xw"]
                    pvT = psum.tile([P, 512], F32, tag="stA", bufs=2)
                    for cc in range(8):
                        nc.tensor.matmul(pvT[:], wv_sb[:, cc, :], xw[:, cc, :],
                                         start=(cc == 0), stop=(cc == 7))
                    vT = small.tile([P, 512], BF16, tag="vT")
                    nc.vector.tensor_copy(vT[:], pvT[:])
                    st["vT"] = vT

                def u4():
                    vT = st["vT"]
                    for j in range(4):
                        pv = psum.tile([P, P], BF16, tag="stA", bufs=2)
                        nc.tensor.transpose(pv[:], vT[:, P * j: P * (j + 1)],
                                            id_sb[:])
                        gc = 4 * tw + j
                        dst = v_sb[:, 130 * gc: 130 * gc + 130].rearrange(
                            "p (h d) -> p h d", h=2, d=65)[:, :, 0:64]
                        src = pv[:, :].rearrange("p (h d) -> p h d", h=2)
                        nc.vector.tensor_copy(dst, src)

                return [u1, u2, u3, u4]

            def proj_norm(phase):
                """Receive side of one AllToAll phase: load + normalize.
                Returns the normalized [P, 8, HTOK] activation tile."""
                ga = acts.tile([P, 8, HTOK], BF16, tag=f"ga{phase}")
                nc.sync.dma_start(
                    ga[:], a2a_out[phase][:, 0:P, :].rearrange("j p t -> p j t"))
                dn = small.tile([8, 2, HTOK], BF16, tag=f"dn{phase}", bufs=1)
                nc.sync.dma_start(dn[:], a2a_out[phase][:, P:P + 2, :])
                dnf32 = small.tile([8, 2, HTOK], F32, tag=f"dnf32{phase}", bufs=1)
                nc.vector.tensor_copy(dnf32[:], dn[:])
                rf32 = small.tile([8, 2, HTOK], F32, tag=f"rf32{phase}", bufs=1)
                cst = RECIP_APPROX_FAST_CONSTS
                nc.vector._custom_dve(RECIPROCAL_APPROX_FAST, out=rf32[:],
                                      in0=dnf32[:], s0=cst["s0"], s1=cst["s1"],
                                      imm2=cst["imm2"])
                rf = small.tile([8, 2, HTOK], BF16, tag=f"rf{phase}", bufs=1)
                nc.vector.tensor_copy(rf[:], rf32[:])
                if debug:
                    nc.sync.dma_start(
                        dbg_ext[phase, 0].rearrange("(j h) t -> j h t", h=2),
                        dnf32[:])
                if do_norm:
                    # broadcast 1/den along partitions via K=8 select
                    # matmuls, then scale ga straight from PSUM
                    for j in range(NCORES):
                        pb = psum.tile([P, HTOK], F32, tag="stA", bufs=2)
                        for h in range(HPC):
                            nc.tensor.matmul(pb[:], sel_sb[:, 2 * j + h, :],
                                             rf[:, h, :],
                                             start=(h == 0), stop=(h == 1))
                        nc.vector.tensor_mul(ga[:, j, :], ga[:, j, :], pb[:])
                return ga

            def proj_group(phase, ga, tc2, nw):
                """One [128 tok, 512 col] block of the output projection."""
                py = psum.tile([P, 512], F32, tag="stA", bufs=2)
                for cc in range(8):
                    nc.tensor.matmul(
                        py[:], ga[:, cc, P * tc2: P * (tc2 + 1)],
                        wp_sb[:, cc, 512 * nw: 512 * (nw + 1)],
                        start=(cc == 0), stop=(cc == 7))
                ys = small.tile([P, 512], F32, tag="ys")
                nc.vector.tensor_copy(ys[:], py[:])
                nc.sync.dma_start(
                    y_ext[HTOK * phase + P * tc2: HTOK * phase + P * (tc2 + 1),
                          512 * nw: 512 * (nw + 1)],
                    ys[:])

            def fire_a2a(phase):
                nc.gpsimd.collective_compute(
                    "AllToAll", mybir.AluOpType.bypass,
                    replica_groups=[list(range(NCORES))],
                    ins=[a2a_in[phase].opt()], outs=[a2a_out[phase].opt()])

            SEQ = [(qw, b) for qw in range(NQW) for b in range(B)]
            LEAD = 2      # iterations of lead for the stage-A weave

            for rep in range(reps):
                ga0 = None
                # prologue: stage A for the first LEAD windows
                for kq in range(min(LEAD, len(SEQ))):
                    qw2, b2 = SEQ[kq]
                    for u in stage_a_units(4 * b2 + qw2):
                        u()

                for k, (qw, b) in enumerate(SEQ):
                    kmax = 4 * qw + 4
                    q0 = Tb * b + 512 * qw
                    r = q0 // TOKS
                    ph = (q0 % TOKS) // HTOK

                    # ---- filler units for this iteration ----
                    units = []
                    if k + LEAD < len(SEQ):
                        qw2, b2 = SEQ[k + LEAD]
                        units += stage_a_units(4 * b2 + qw2)
                    if NPH == 2 and qw == NQW - 1 and b >= B - 2:
                        # C(phase 0) woven into the last two iterations
                        if b == B - 2:
                            def mknorm():
                                nonlocal ga0
                                ga0 = proj_norm(0)
                            units.append(mknorm)
                            units += [
                                (lambda t=t, n=n: proj_group(0, ga0, t, n))
                                for t in range(2) for n in range(NNW)]
                        else:
                            units += [
                                (lambda t=t, n=n: proj_group(0, ga0, t, n))
                                for t in range(2, 4) for n in range(NNW)]

                    po = [psum.tile([P, 512], F32, tag="po", bufs=2,
                                    name=f"po{k}_{lh_}")
                          for lh_ in range(HPC)]
                    pts = []

                    def pv(kc):
                        pt, c0 = pts[kc]
                        gc = (Tb // 128) * b + kc
                        for lh in range(HPC):
                            nc.tensor.matmul(
                                po[lh][0:65, c0:512],
                                v_sb[:, 130 * gc + 65 * lh:
                                     130 * gc + 65 * lh + 65],
                                pt[:, 512 * lh + c0: 512 * lh + 512],
                                start=(kc == 0), stop=(kc == kmax - 1),
                                skip_group_check=True)

                    emitted = 0
                    for kc in range(kmax):
                        k0 = Tb * b + P * kc
                        j = kc - 4 * qw
                        c0 = max(0, j) * P
                        ps = psum.tile([P, 1024], F32, tag="pair", bufs=2)
                        pt = ptiles.tile([P, 1024], BF16, tag="pT")
                        pts.append((pt, c0))
                        for lh in range(HPC):
                            hs = 64 * lh
                            nc.tensor.matmul(
                                ps[:, 512 * lh + c0: 512 * lh + 512],
                                kT_sb[hs: hs + 64, k0: k0 + P],
                                qT_sb[hs: hs + 64, q0 + c0: q0 + 512],
                                start=True, stop=True)
                        # one exp for both heads; rank-3 AP skips the dead
                        # prefix of diagonal chunks
                        src = ps[:].rearrange("p (h x) -> p h x", h=2)
                        dst = pt[:].rearrange("p (h x) -> p h x", h=2)
                        nc.scalar.activation(
                            dst[:, :, c0:512], src[:, :, c0:512],
                            mybir.ActivationFunctionType.Exp,
                            scale=float(SCALE))
                        if j >= 0:
                            nc.vector.tensor_mul(
                                dst[:, :, c0: c0 + P],
                                dst[:, :, c0: c0 + P],
                                mk_sb[:].rearrange("p (h x) -> p h x", h=2))
                        # weave filler, then the lag-1 PV
                        target = ((kc + 1) * len(units)) // kmax
                        while emitted < target:
                            units[emitted]()
                            emitted += 1
                        if kc >= 1:
                            pv(kc - 1)
                    pv(kmax - 1)

                    for lh in range(HPC):
                        oa = small.tile([65, 512], BF16, tag="oa")
                        nc.vector.tensor_copy(oa[:], po[lh][0:65, :])
                        off = (q0 % TOKS) % HTOK
                        nc.sync.dma_start(
                            a2a_in[ph][r, 64 * lh: 64 * lh + 64, off: off + 512],
                            oa[0:64, :])
                        nc.sync.dma_start(
                            a2a_in[ph][r, P + lh, off: off + 512],
                            oa[64:65, :])

                    if NPH == 2 and k == B * (NQW - 2) + B - 1:
                        fire_a2a(0)

                # final phase: collective + projection
                fire_a2a(NPH - 1)
                ga1 = proj_norm(NPH - 1)
                for tc2 in range(HTOK // P):
                    for nw in range(NNW):
                        proj_group(NPH - 1, ga1, tc2, nw)

    nc.finalize()
    return nc


def _host_inputs(x, w_attn, w_proj, Tb=T):
    import ml_dtypes
    bf16 = ml_dtypes.bfloat16
    BT = B * Tb
    xT = np.ascontiguousarray(x.reshape(BT, C).T).astype(bf16)
    wproj_bf = np.ascontiguousarray(w_proj).astype(bf16)
    rr = np.arange(P)[:, None]
    cc = np.arange(P)[None, :]
    tri = (rr <= cc).astype(bf16)
    masks = np.concatenate([tri, tri], axis=1)   # [128, 256]
    ident = np.eye(P).astype(bf16)
    ii = np.arange(8)[:, None, None]
    ss = np.arange(16)[None, :, None]     # slot = 2j + h
    pp = np.arange(P)[None, None, :]
    sel = ((ii == ss // 2) & (pp // 64 == ss % 2)).astype(bf16)  # [8, 16, 128]
    in_maps = []
    for g in range(NCORES):
        in_maps.append({
            "xT": xT,
            "wq": np.ascontiguousarray(w_attn[:, DH * g: DH * (g + 1)]).astype(bf16),
            "wk": np.ascontiguousarray(w_attn[:, C + DH * g: C + DH * (g + 1)]).astype(bf16),
            "wv": np.ascontiguousarray(w_attn[:, 2 * C + DH * g: 2 * C + DH * (g + 1)]).astype(bf16),
            "wproj": wproj_bf,
            "masks": masks,
            "sel": sel,
            "ident": ident,
        })
    return in_maps


_NC_CACHE = {}


def kernel(x, w_attn, w_proj):
    x = np.asarray(x)
    w_attn = np.asarray(w_attn)
    w_proj = np.asarray(w_proj)
    if T not in _NC_CACHE:
        _NC_CACHE[T] = build_nc(T)
    nc = _NC_CACHE[T]
    in_maps = _host_inputs(x, w_attn, w_proj, T)
    res = run_bass_kernel_spmd(nc, in_maps, core_ids=list(range(NCORES)))
    y = np.concatenate([res.results[g]["y"] for g in range(NCORES)], axis=0)
    return y.reshape(B, T, C).astype(np.float32)


# revision 35
# speedup vs baseline: 1.3614x; 1.3614x over previous
"""Distributed causal self-attention for 8 TRN2 NeuronCores (v2).

Sharding: tensor-parallel over heads (2 heads/core, all batches); an
AllToAll (split in 2 phases) redistributes the attention output from
head-sharded to token-sharded for the output projection.

v2 structure (per core, all matmuls bf16, fp32 PSUM):
  - qw-major rounds interleaving QKV projection (stage A) with attention
    (stage B): round r computes scores/exp/PV for query window r of all
    batches while the PE also runs stage-A matmuls for window r+1, so the
    ACT engine's exp stream hides under PE work.
  - scores for the two heads are emitted back-to-back as 64-row PE tiles
    (lhsT/rhs base partitions 0 and 64) -> they run concurrently in the
    PE array (row tiling).
  - per key-chunk PSUM tile [128, 1024]: cols 0:512 head0, 512:1024
    head1; ONE exp instruction per chunk with a rank-3 AP that skips the
    causally-dead prefix of diagonal chunks.  Causal triangle masked by a
    bf16 0/1 multiply on the [128,128] diagonal tiles only.
  - PV accumulates O^T (rows 0:64) and the softmax denominator (row 64,
    via a ones column baked into v_sb) in PSUM; the UNNORMALIZED output
    plus denominator rows travel through the AllToAll ([8, 130, 512]
    payload) and normalization happens on the receive side: one fast
    reciprocal + gpsimd partition-broadcasts + one big DVE multiply per
    phase.
"""

import numpy as np

import concourse.bass as bass
import concourse.bacc as bacc
import concourse.mybir as mybir
import concourse.tile as tile
from concourse.bass_utils import run_bass_kernel_spmd
from concourse.dve_ops import RECIPROCAL_APPROX_FAST, RECIP_APPROX_FAST_CONSTS

B, T, C = 4, 2048, 1024
H, D = 16, 64
NCORES = 8
HPC = H // NCORES        # heads per core
DH = HPC * D             # 128 qkv cols per core
P = 128
F32 = mybir.dt.float32
BF16 = mybir.dt.bfloat16
SCALE = 1.0 / np.sqrt(D)


def build_nc(Tb=T, reps=1, debug=False, do_norm=True):
    BT = B * Tb              # total tokens
    NTW = BT // 512          # 512-token windows (stage A units)
    NQW = Tb // 512          # query windows per batch
    NCH = BT // 128          # 128-token chunks total
    TOKS = BT // NCORES      # tokens per core in the proj stage
    NNW = C // 512           # output column windows
    NPH = 2 if Tb >= 2048 else 1
    HTOK = TOKS // NPH       # tokens per proj phase

    nc = bacc.Bacc(None, target_bir_lowering=False)

    xT_ext = nc.declare_dram_parameter("xT", [C, BT], BF16, isOutput=False)
    wq_ext = nc.declare_dram_parameter("wq", [C, DH], BF16, isOutput=False)
    wk_ext = nc.declare_dram_parameter("wk", [C, DH], BF16, isOutput=False)
    wv_ext = nc.declare_dram_parameter("wv", [C, DH], BF16, isOutput=False)
    wp_ext = nc.declare_dram_parameter("wproj", [C, C], BF16, isOutput=False)
    mk_ext = nc.declare_dram_parameter("masks", [P, 256], BF16, isOutput=False)
    sel_ext = nc.declare_dram_parameter("sel", [8, 16, P], BF16, isOutput=False)
    id_ext = nc.declare_dram_parameter("ident", [P, P], BF16, isOutput=False)
    y_ext = nc.declare_dram_parameter("y", [TOKS, C], F32, isOutput=True)
    dbg_ext = (nc.declare_dram_parameter("dbg", [NPH, 2, 16, HTOK], F32,
                                         isOutput=True) if debug else None)
    dbg2_ext = (nc.declare_dram_parameter("dbg2", [3, P, 1024], F32,
                                          isOutput=True) if debug else None)
    dbg3_ext = (nc.declare_dram_parameter("dbg3", [2, 130, 512], BF16,
                                          isOutput=True) if debug else None)
    dbg4_ext = (nc.declare_dram_parameter("dbg4", [P, 8, HTOK], BF16,
                                          isOutput=True) if debug else None)

    xT_v = xT_ext.rearrange("(c p) t -> p c t", p=P)     # [128, 8, BT]
    wq_v = wq_ext.rearrange("(c p) m -> p c m", p=P)
    wk_v = wk_ext.rearrange("(c p) m -> p c m", p=P)
    wv_v = wv_ext.rearrange("(c p) m -> p c m", p=P)
    wp_v = wp_ext.rearrange("(c p) m -> p c m", p=P)     # [128, 8, 1024]

    with tile.TileContext(nc, num_cores=NCORES) as tc:
        with (
            tc.tile_pool(name="consts", bufs=1) as consts,
            tc.tile_pool(name="acts", bufs=1) as acts,
            tc.tile_pool(name="xin", bufs=3) as xin,
            tc.tile_pool(name="small", bufs=4) as small,
            tc.tile_pool(name="ptiles", bufs=6) as ptiles,
            tc.tile_pool(name="psum", bufs=1, space="PSUM") as psum,
            tc.tile_pool(name="dram", bufs=1, space="DRAM") as dram,
        ):
            # ---- constants ----
            wq_sb = consts.tile([P, 8, DH], BF16)
            wk_sb = consts.tile([P, 8, DH], BF16)
            wv_sb = consts.tile([P, 8, DH], BF16)
            wp_sb = consts.tile([P, 8, C], BF16)
            mk_sb = consts.tile([P, 256], BF16)   # [tri | tri]
            id_sb = consts.tile([P, P], BF16)
            sel_sb = consts.tile([8, 16, P], BF16)
            nc.gpsimd.dma_start(sel_sb[:], sel_ext[:])
            nc.gpsimd.dma_start(wq_sb[:], wq_v[:])
            nc.gpsimd.dma_start(wk_sb[:], wk_v[:])
            nc.gpsimd.dma_start(wv_sb[:], wv_v[:])
            nc.gpsimd.dma_start(wp_sb[:], wp_v[:])
            nc.gpsimd.dma_start(mk_sb[:], mk_ext[:])
            nc.gpsimd.dma_start(id_sb[:], id_ext[:])

            # ---- persistent activations ----
            qT_sb = acts.tile([P, BT], BF16)
            kT_sb = acts.tile([P, BT], BF16)
            v_sb = acts.tile([P, 130 * NCH], BF16)
            nc.vector.memset(v_sb[:], 1.0)  # bakes in the ones columns

            a2a_in = [dram.tile([NCORES, 130, HTOK], BF16, name=f"a2ain{p}",
                                tag=f"a2ain{p}") for p in range(NPH)]
            a2a_out = [dram.tile([NCORES, 130, HTOK], BF16, name=f"a2aout{p}",
                                 tag=f"a2aout{p}") for p in range(NPH)]

            def fetch_x(tw):
                xw = xin.tile([P, 8, 512], BF16, tag="xw")
                nc.sync.dma_start(xw[:], xT_v[:, :, 512 * tw: 512 * (tw + 1)])
                return xw

            def stage_a_units(tw, xw):
                """QKV projection for one 512-token window, split into 4
                filler units (~1-2us of PE work each); xw was prefetched."""
                st = {}

                def u1():
                    pq = psum.tile([P, 512], F32, tag="stA", bufs=2)
                    for cc in range(8):
                        nc.tensor.matmul(pq[:], wq_sb[:, cc, :], xw[:, cc, :],
                                         start=(cc == 0), stop=(cc == 7))
                    nc.vector.tensor_copy(qT_sb[:, 512 * tw: 512 * (tw + 1)], pq[:])

                def u2():
                    pk = psum.tile([P, 512], F32, tag="stA", bufs=2)
                    for cc in range(8):
                        nc.tensor.matmul(pk[:], wk_sb[:, cc, :], xw[:, cc, :],
                                         start=(cc == 0), stop=(cc == 7))
                    nc.vector.tensor_copy(kT_sb[:, 512 * tw: 512 * (tw + 1)], pk[:])

                def u3():
                    pvT = psum.tile([P, 512], F32, tag="stA", bufs=2)
                    for cc in range(8):
                        nc.tensor.matmul(pvT[:], wv_sb[:, cc, :], xw[:, cc, :],
                                         start=(cc == 0), stop=(cc == 7))
                    vT = small.tile([P, 512], BF16, tag="vT")
                    nc.vector.tensor_copy(vT[:], pvT[:])
                    st["vT"] = vT

                def u4():
                    vT = st["vT"]
                    for j in range(4):
                        pv = psum.tile([P, P], BF16, tag="stA", bufs=2)
                        nc.tensor.transpose(pv[:], vT[:, P * j: P * (j + 1)],
                                            id_sb[:])
                        gc = 4 * tw + j
                        dst = v_sb[:, 130 * gc: 130 * gc + 130].rearrange(
                            "p (h d) -> p h d", h=2, d=65)[:, :, 0:64]
                        src = pv[:, :].rearrange("p (h d) -> p h d", h=2)
                        nc.vector.tensor_copy(dst, src)

                return [u1, u2, u3, u4]

            def proj_norm(phase):
                """Receive side of one AllToAll phase: load + normalize.
                Returns the normalized [P, 8, HTOK] activation tile."""
                ga = acts.tile([P, 8, HTOK], BF16, tag=f"ga{phase}")
                nc.sync.dma_start(
                    ga[:], a2a_out[phase][:, 0:P, :].rearrange("j p t -> p j t"))
                dn = small.tile([8, 2, HTOK], BF16, tag=f"dn{phase}", bufs=1)
                nc.sync.dma_start(dn[:], a2a_out[phase][:, P:P + 2, :])
                dnf32 = small.tile([8, 2, HTOK], F32, tag=f"dnf32{phase}", bufs=1)
                nc.vector.tensor_copy(dnf32[:], dn[:])
                rf32 = small.tile([8, 2, HTOK], F32, tag=f"rf32{phase}", bufs=1)
                cst = RECIP_APPROX_FAST_CONSTS
                nc.vector._custom_dve(RECIPROCAL_APPROX_FAST, out=rf32[:],
                                      in0=dnf32[:], s0=cst["s0"], s1=cst["s1"],
                                      imm2=cst["imm2"])
                rf = small.tile([8, 2, HTOK], BF16, tag=f"rf{phase}", bufs=1)
                nc.vector.tensor_copy(rf[:], rf32[:])
                if debug:
                    nc.sync.dma_start(
                        dbg_ext[phase, 0].rearrange("(j h) t -> j h t", h=2),
                        dnf32[:])
                if do_norm:
                    # broadcast 1/den along partitions via K=8 select
                    # matmuls, then scale ga straight from PSUM
                    for j in range(NCORES):
                        pb = psum.tile([P, HTOK], F32, tag="stA", bufs=2)
                        for h in range(HPC):
                            nc.tensor.matmul(pb[:], sel_sb[:, 2 * j + h, :],
                                             rf[:, h, :],
                                             start=(h == 0), stop=(h == 1))
                        nc.vector.tensor_mul(ga[:, j, :], ga[:, j, :], pb[:])
                return ga

            def proj_group(phase, ga, tc2, nw):
                """One [128 tok, 512 col] block of the output projection."""
                py = psum.tile([P, 512], F32, tag="stA", bufs=2)
                for cc in range(8):
                    nc.tensor.matmul(
                        py[:], ga[:, cc, P * tc2: P * (tc2 + 1)],
                        wp_sb[:, cc, 512 * nw: 512 * (nw + 1)],
                        start=(cc == 0), stop=(cc == 7))
                ys = small.tile([P, 512], F32, tag="ys")
                nc.vector.tensor_copy(ys[:], py[:])
                nc.sync.dma_start(
                    y_ext[HTOK * phase + P * tc2: HTOK * phase + P * (tc2 + 1),
                          512 * nw: 512 * (nw + 1)],
                    ys[:])

            def fire_a2a(phase):
                nc.gpsimd.collective_compute(
                    "AllToAll", mybir.AluOpType.bypass,
                    replica_groups=[list(range(NCORES))],
                    ins=[a2a_in[phase].opt()], outs=[a2a_out[phase].opt()])

            SEQ = [(qw, b) for qw in range(NQW) for b in range(B)]

            for rep in range(reps):
                ga0 = None
                # prologue: stage A for the first window, prefetch the next
                xw_pend = {}
                qw2, b2 = SEQ[0]
                xw0 = fetch_x(4 * b2 + qw2)
                for u in stage_a_units(4 * b2 + qw2, xw0):
                    u()
                if len(SEQ) > 1:
                    qw2, b2 = SEQ[1]
                    xw_pend[1] = fetch_x(4 * b2 + qw2)

                for k, (qw, b) in enumerate(SEQ):
                    kmax = 4 * qw + 4
                    q0 = Tb * b + 512 * qw
                    r = q0 // TOKS
                    ph = (q0 % TOKS) // HTOK

                    # prefetch x for the window woven NEXT iteration
                    if k + 2 < len(SEQ):
                        qw2, b2 = SEQ[k + 2]
                        xw_pend[k + 2] = fetch_x(4 * b2 + qw2)
                    # ---- filler units for this iteration ----
                    units = []
                    if k + 1 < len(SEQ):
                        qw2, b2 = SEQ[k + 1]
                        units += stage_a_units(4 * b2 + qw2, xw_pend.pop(k + 1))
                    if NPH == 2 and qw == NQW - 1 and b >= B - 2:
                        # C(phase 0) woven into the last two iterations
                        if b == B - 2:
                            def mknorm():
                                nonlocal ga0
                                ga0 = proj_norm(0)
                            units.append(mknorm)
                            units += [
                                (lambda t=t, n=n: proj_group(0, ga0, t, n))
                                for t in range(2) for n in range(NNW)]
                        else:
                            units += [
                                (lambda t=t, n=n: proj_group(0, ga0, t, n))
                                for t in range(2, 4) for n in range(NNW)]

                    po = [psum.tile([P, 512], F32, tag="po", bufs=2,
                                    name=f"po{k}_{lh_}")
                          for lh_ in range(HPC)]
                    pts = []

                    def pv(kc):
                        pt, c0 = pts[kc]
                        gc = (Tb // 128) * b + kc
                        for lh in range(HPC):
                            nc.tensor.matmul(
                                po[lh][0:65, c0:512],
                                v_sb[:, 130 * gc + 65 * lh:
                                     130 * gc + 65 * lh + 65],
                                pt[:, 512 * lh + c0: 512 * lh + 512],
                                start=(kc == 0), stop=(kc == kmax - 1),
                                skip_group_check=True)

                    emitted = 0
                    for kc in range(kmax):
                        k0 = Tb * b + P * kc
                        j = kc - 4 * qw
                        c0 = max(0, j) * P
                        ps = psum.tile([P, 1024], F32, tag="pair", bufs=2)
                        pt = ptiles.tile([P, 1024], BF16, tag="pT")
                        pts.append((pt, c0))
                        for lh in range(HPC):
                            hs = 64 * lh
                            nc.tensor.matmul(
                                ps[:, 512 * lh + c0: 512 * lh + 512],
                                kT_sb[hs: hs + 64, k0: k0 + P],
                                qT_sb[hs: hs + 64, q0 + c0: q0 + 512],
                                start=True, stop=True)
                        # one exp for both heads; rank-3 AP skips the dead
                        # prefix of diagonal chunks
                        src = ps[:].rearrange("p (h x) -> p h x", h=2)
                        dst = pt[:].rearrange("p (h x) -> p h x", h=2)
                        nc.scalar.activation(
                            dst[:, :, c0:512], src[:, :, c0:512],
                            mybir.ActivationFunctionType.Exp,
                            scale=float(SCALE))
                        if j >= 0:
                            nc.vector.tensor_mul(
                                dst[:, :, c0: c0 + P],
                                dst[:, :, c0: c0 + P],
                                mk_sb[:].rearrange("p (h x) -> p h x", h=2))
                        # weave filler, then the lag-1 PV
                        target = ((kc + 1) * len(units)) // kmax
                        while emitted < target:
                            units[emitted]()
                            emitted += 1
                        if kc >= 1:
                            pv(kc - 1)
                    pv(kmax - 1)

                    for lh in range(HPC):
                        oa = small.tile([65, 512], BF16, tag="oa")
                        nc.vector.tensor_copy(oa[:], po[lh][0:65, :])
                        off = (q0 % TOKS) % HTOK
                        nc.sync.dma_start(
                            a2a_in[ph][r, 64 * lh: 64 * lh + 64, off: off + 512],
                            oa[0:64, :])
                        nc.sync.dma_start(
                            a2a_in[ph][r, P + lh, off: off + 512],
                            oa[64:65, :])

                    if NPH == 2 and k == B * (NQW - 2) + B - 1:
                        fire_a2a(0)

                # final phase: collective + projection
                fire_a2a(NPH - 1)
                ga1 = proj_norm(NPH - 1)
                for tc2 in range(HTOK // P):
                    for nw in range(NNW):
                        proj_group(NPH - 1, ga1, tc2, nw)

    nc.finalize()
    return nc


def _host_inputs(x, w_attn, w_proj, Tb=T):
    import ml_dtypes
    bf16 = ml_dtypes.bfloat16
    BT = B * Tb
    xT = np.ascontiguousarray(x.reshape(BT, C).T).astype(bf16)
    wproj_bf = np.ascontiguousarray(w_proj).astype(bf16)
    rr = np.arange(P)[:, None]
    cc = np.arange(P)[None, :]
    tri = (rr <= cc).astype(bf16)
    masks = np.concatenate([tri, tri], axis=1)   # [128, 256]
    ident = np.eye(P).astype(bf16)
    ii = np.arange(8)[:, None, None]
    ss = np.arange(16)[None, :, None]     # slot = 2j + h
    pp = np.arange(P)[None, None, :]
    sel = ((ii == ss // 2) & (pp // 64 == ss % 2)).astype(bf16)  # [8, 16, 128]
    in_maps = []
    for g in range(NCORES):
        in_maps.append({
            "xT": xT,
            "wq": np.ascontiguousarray(w_attn[:, DH * g: DH * (g + 1)]).astype(bf16),
            "wk": np.ascontiguousarray(w_attn[:, C + DH * g: C + DH * (g + 1)]).astype(bf16),
            "wv": np.ascontiguousarray(w_attn[:, 2 * C + DH * g: 2 * C + DH * (g + 1)]).astype(bf16),
            "wproj": wproj_bf,
            "masks": masks,
            "sel": sel,
            "ident": ident,
        })
    return in_maps


_NC_CACHE = {}


def kernel(x, w_attn, w_proj):
    x = np.asarray(x)
    w_attn = np.asarray(w_attn)
    w_proj = np.asarray(w_proj)
    if T not in _NC_CACHE:
        _NC_CACHE[T] = build_nc(T)
    nc = _NC_CACHE[T]
    in_maps = _host_inputs(x, w_attn, w_proj, T)
    res = run_bass_kernel_spmd(nc, in_maps, core_ids=list(range(NCORES)))
    y = np.concatenate([res.results[g]["y"] for g in range(NCORES)], axis=0)
    return y.reshape(B, T, C).astype(np.float32)


# revision 36
# speedup vs baseline: 1.7167x; 1.2609x over previous
"""Distributed causal self-attention for 8 TRN2 NeuronCores (v2).

Sharding: tensor-parallel over heads (2 heads/core, all batches); an
AllToAll (split in 2 phases) redistributes the attention output from
head-sharded to token-sharded for the output projection.

v2 structure (per core, all matmuls bf16, fp32 PSUM):
  - qw-major rounds interleaving QKV projection (stage A) with attention
    (stage B): round r computes scores/exp/PV for query window r of all
    batches while the PE also runs stage-A matmuls for window r+1, so the
    ACT engine's exp stream hides under PE work.
  - scores for the two heads are emitted back-to-back as 64-row PE tiles
    (lhsT/rhs base partitions 0 and 64) -> they run concurrently in the
    PE array (row tiling).
  - per key-chunk PSUM tile [128, 1024]: cols 0:512 head0, 512:1024
    head1; ONE exp instruction per chunk with a rank-3 AP that skips the
    causally-dead prefix of diagonal chunks.  Causal triangle masked by a
    bf16 0/1 multiply on the [128,128] diagonal tiles only.
  - PV accumulates O^T (rows 0:64) and the softmax denominator (row 64,
    via a ones column baked into v_sb) in PSUM; the UNNORMALIZED output
    plus denominator rows travel through the AllToAll ([8, 130, 512]
    payload) and normalization happens on the receive side: one fast
    reciprocal + gpsimd partition-broadcasts + one big DVE multiply per
    phase.
"""

import numpy as np

import concourse.bass as bass
import concourse.bacc as bacc
import concourse.mybir as mybir
import concourse.tile as tile
from concourse.bass_utils import run_bass_kernel_spmd
from concourse.dve_ops import RECIPROCAL_APPROX_FAST, RECIP_APPROX_FAST_CONSTS

B, T, C = 4, 2048, 1024
H, D = 16, 64
NCORES = 8
HPC = H // NCORES        # heads per core
DH = HPC * D             # 128 qkv cols per core
P = 128
F32 = mybir.dt.float32
BF16 = mybir.dt.bfloat16
SCALE = 1.0 / np.sqrt(D)


def build_nc(Tb=T, reps=1, debug=False, do_norm=True):
    BT = B * Tb              # total tokens
    NTW = BT // 512          # 512-token windows (stage A units)
    NQW = Tb // 512          # query windows per batch
    NCH = BT // 128          # 128-token chunks total
    TOKS = BT // NCORES      # tokens per core in the proj stage
    NNW = C // 512           # output column windows
    NPH = 2 if Tb >= 2048 else 1
    HTOK = TOKS // NPH       # tokens per proj phase

    nc = bacc.Bacc(None, target_bir_lowering=False)

    xT_ext = nc.declare_dram_parameter("xT", [C, BT], BF16, isOutput=False)
    wq_ext = nc.declare_dram_parameter("wq", [C, DH], BF16, isOutput=False)
    wk_ext = nc.declare_dram_parameter("wk", [C, DH], BF16, isOutput=False)
    wv_ext = nc.declare_dram_parameter("wv", [C, DH], BF16, isOutput=False)
    wp_ext = nc.declare_dram_parameter("wproj", [C, C], BF16, isOutput=False)
    mk_ext = nc.declare_dram_parameter("masks", [P, 256], BF16, isOutput=False)
    sel_ext = nc.declare_dram_parameter("sel", [16, 8, P], BF16, isOutput=False)
    id_ext = nc.declare_dram_parameter("ident", [P, P], BF16, isOutput=False)
    y_ext = nc.declare_dram_parameter("y", [TOKS, C], F32, isOutput=True)
    dbg_ext = (nc.declare_dram_parameter("dbg", [NPH, 2, 16, HTOK], F32,
                                         isOutput=True) if debug else None)
    dbg2_ext = (nc.declare_dram_parameter("dbg2", [3, P, 1024], F32,
                                          isOutput=True) if debug else None)
    dbg3_ext = (nc.declare_dram_parameter("dbg3", [2, 130, 512], BF16,
                                          isOutput=True) if debug else None)
    dbg4_ext = (nc.declare_dram_parameter("dbg4", [P, 8, HTOK], BF16,
                                          isOutput=True) if debug else None)

    xT_v = xT_ext.rearrange("(c p) t -> p c t", p=P)     # [128, 8, BT]
    wq_v = wq_ext.rearrange("(c p) m -> p c m", p=P)
    wk_v = wk_ext.rearrange("(c p) m -> p c m", p=P)
    wv_v = wv_ext.rearrange("(c p) m -> p c m", p=P)
    wp_v = wp_ext.rearrange("(c p) m -> p c m", p=P)     # [128, 8, 1024]

    with tile.TileContext(nc, num_cores=NCORES) as tc:
        with (
            tc.tile_pool(name="consts", bufs=1) as consts,
            tc.tile_pool(name="acts", bufs=1) as acts,
            tc.tile_pool(name="xin", bufs=3) as xin,
            tc.tile_pool(name="small", bufs=4) as small,
            tc.tile_pool(name="ptiles", bufs=6) as ptiles,
            tc.tile_pool(name="psum", bufs=1, space="PSUM") as psum,
            tc.tile_pool(name="dram", bufs=1, space="DRAM") as dram,
        ):
            # ---- constants ----
            wq_sb = consts.tile([P, 8, DH], BF16)
            wk_sb = consts.tile([P, 8, DH], BF16)
            wv_sb = consts.tile([P, 8, DH], BF16)
            wp_sb = consts.tile([P, 8, C], BF16)
            mk_sb = consts.tile([P, 256], BF16)   # [tri | tri]
            id_sb = consts.tile([P, P], BF16)
            sel_sb = consts.tile([16, 8, P], BF16)
            nc.gpsimd.dma_start(sel_sb[:], sel_ext[:])
            nc.gpsimd.dma_start(wq_sb[:], wq_v[:])
            nc.gpsimd.dma_start(wk_sb[:], wk_v[:])
            nc.gpsimd.dma_start(wv_sb[:], wv_v[:])
            nc.gpsimd.dma_start(wp_sb[:], wp_v[:])
            nc.gpsimd.dma_start(mk_sb[:], mk_ext[:])
            nc.gpsimd.dma_start(id_sb[:], id_ext[:])

            # ---- persistent activations ----
            qT_sb = acts.tile([P, BT], BF16)
            kT_sb = acts.tile([P, BT], BF16)
            v_sb = acts.tile([P, 130 * NCH], BF16)
            nc.vector.memset(v_sb[:], 1.0)  # bakes in the ones columns

            a2a_in = [dram.tile([NCORES, 130, HTOK], BF16, name=f"a2ain{p}",
                                tag=f"a2ain{p}") for p in range(NPH)]
            a2a_out = [dram.tile([NCORES, 130, HTOK], BF16, name=f"a2aout{p}",
                                 tag=f"a2aout{p}") for p in range(NPH)]

            def fetch_x(tw):
                xw = xin.tile([P, 8, 512], BF16, tag="xw")
                nc.sync.dma_start(xw[:], xT_v[:, :, 512 * tw: 512 * (tw + 1)])
                return xw

            def stage_a_units(tw, xw):
                """QKV projection for one 512-token window, split into 4
                filler units (~1-2us of PE work each); xw was prefetched."""
                st = {}

                def u1():
                    pq = psum.tile([P, 512], F32, tag="stA", bufs=2)
                    for cc in range(8):
                        nc.tensor.matmul(pq[:], wq_sb[:, cc, :], xw[:, cc, :],
                                         start=(cc == 0), stop=(cc == 7))
                    nc.vector.tensor_copy(qT_sb[:, 512 * tw: 512 * (tw + 1)], pq[:])

                def u2():
                    pk = psum.tile([P, 512], F32, tag="stA", bufs=2)
                    for cc in range(8):
                        nc.tensor.matmul(pk[:], wk_sb[:, cc, :], xw[:, cc, :],
                                         start=(cc == 0), stop=(cc == 7))
                    nc.vector.tensor_copy(kT_sb[:, 512 * tw: 512 * (tw + 1)], pk[:])

                def u3():
                    pvT = psum.tile([P, 512], F32, tag="stA", bufs=2)
                    for cc in range(8):
                        nc.tensor.matmul(pvT[:], wv_sb[:, cc, :], xw[:, cc, :],
                                         start=(cc == 0), stop=(cc == 7))
                    vT = small.tile([P, 512], BF16, tag="vT")
                    nc.vector.tensor_copy(vT[:], pvT[:])
                    st["vT"] = vT

                def u4():
                    vT = st["vT"]
                    for j in range(4):
                        pv = psum.tile([P, P], BF16, tag="stA", bufs=2)
                        nc.tensor.transpose(pv[:], vT[:, P * j: P * (j + 1)],
                                            id_sb[:])
                        gc = 4 * tw + j
                        dst = v_sb[:, 130 * gc: 130 * gc + 130].rearrange(
                            "p (h d) -> p h d", h=2, d=65)[:, :, 0:64]
                        src = pv[:, :].rearrange("p (h d) -> p h d", h=2)
                        nc.vector.tensor_copy(dst, src)

                return [u1, u2, u3, u4]

            def proj_norm(phase):
                """Receive side of one AllToAll phase: load + normalize.
                Returns the normalized [P, 8, HTOK] activation tile."""
                ga = acts.tile([P, 8, HTOK], BF16, tag=f"ga{phase}")
                nc.sync.dma_start(
                    ga[:], a2a_out[phase][:, 0:P, :].rearrange("j p t -> p j t"))
                # denominators h-major: rows 0:8 = head0 of cores 0..7,
                # rows 8:16 = head1 (two clean DMAs, no partition split)
                dn = small.tile([16, HTOK], BF16, tag=f"dn{phase}", bufs=1)
                nc.sync.dma_start(dn[0:8, :], a2a_out[phase][:, P, :])
                nc.sync.dma_start(dn[8:16, :], a2a_out[phase][:, P + 1, :])
                dnf32 = small.tile([16, HTOK], F32, tag=f"dnf32{phase}", bufs=1)
                nc.vector.tensor_copy(dnf32[:], dn[:])
                rf32 = small.tile([16, HTOK], F32, tag=f"rf32{phase}", bufs=1)
                cst = RECIP_APPROX_FAST_CONSTS
                nc.vector._custom_dve(RECIPROCAL_APPROX_FAST, out=rf32[:],
                                      in0=dnf32[:], s0=cst["s0"], s1=cst["s1"],
                                      imm2=cst["imm2"])
                rf = small.tile([16, HTOK], BF16, tag=f"rf{phase}", bufs=1)
                nc.vector.tensor_copy(rf[:], rf32[:])
                if debug:
                    nc.sync.dma_start(
                        dbg_ext[phase, 0].rearrange("(j h) t -> j h t", h=2),
                        dnf32[:])
                if do_norm:
                    # broadcast 1/den along partitions via one K=16 select
                    # matmul per source core, then scale ga from PSUM
                    for j in range(NCORES):
                        pb = psum.tile([P, HTOK], F32, tag="stA", bufs=2)
                        nc.tensor.matmul(pb[:], sel_sb[:, j, :], rf[:],
                                         start=True, stop=True)
                        nc.vector.tensor_mul(ga[:, j, :], ga[:, j, :], pb[:])
                return ga

            def proj_group(phase, ga, tc2, nw):
                """One [128 tok, 512 col] block of the output projection."""
                py = psum.tile([P, 512], F32, tag="stA", bufs=2)
                for cc in range(8):
                    nc.tensor.matmul(
                        py[:], ga[:, cc, P * tc2: P * (tc2 + 1)],
                        wp_sb[:, cc, 512 * nw: 512 * (nw + 1)],
                        start=(cc == 0), stop=(cc == 7))
                ys = small.tile([P, 512], F32, tag="ys")
                nc.vector.tensor_copy(ys[:], py[:])
                nc.sync.dma_start(
                    y_ext[HTOK * phase + P * tc2: HTOK * phase + P * (tc2 + 1),
                          512 * nw: 512 * (nw + 1)],
                    ys[:])

            def fire_a2a(phase):
                nc.gpsimd.collective_compute(
                    "AllToAll", mybir.AluOpType.bypass,
                    replica_groups=[list(range(NCORES))],
                    ins=[a2a_in[phase].opt()], outs=[a2a_out[phase].opt()])

            SEQ = [(qw, b) for qw in range(NQW) for b in range(B)]

            for rep in range(reps):
                ga0 = None
                # prologue: stage A for the first window, prefetch the next
                xw_pend = {}
                qw2, b2 = SEQ[0]
                xw0 = fetch_x(4 * b2 + qw2)
                for u in stage_a_units(4 * b2 + qw2, xw0):
                    u()
                if len(SEQ) > 1:
                    qw2, b2 = SEQ[1]
                    xw_pend[1] = fetch_x(4 * b2 + qw2)

                for k, (qw, b) in enumerate(SEQ):
                    kmax = 4 * qw + 4
                    q0 = Tb * b + 512 * qw
                    r = q0 // TOKS
                    ph = (q0 % TOKS) // HTOK

                    # prefetch x for the window woven NEXT iteration
                    if k + 2 < len(SEQ):
                        qw2, b2 = SEQ[k + 2]
                        xw_pend[k + 2] = fetch_x(4 * b2 + qw2)
                    # ---- filler units for this iteration ----
                    units = []
                    if k + 1 < len(SEQ):
                        qw2, b2 = SEQ[k + 1]
                        units += stage_a_units(4 * b2 + qw2, xw_pend.pop(k + 1))
                    if NPH == 2 and qw == NQW - 1 and b >= B - 2:
                        # C(phase 0) woven into the last two iterations
                        if b == B - 2:
                            def mknorm():
                                nonlocal ga0
                                ga0 = proj_norm(0)
                            units.append(mknorm)
                            units += [
                                (lambda t=t, n=n: proj_group(0, ga0, t, n))
                                for t in range(2) for n in range(NNW)]
                        else:
                            units += [
                                (lambda t=t, n=n: proj_group(0, ga0, t, n))
                                for t in range(2, 4) for n in range(NNW)]

                    po = [psum.tile([P, 512], F32, tag="po", bufs=2,
                                    name=f"po{k}_{lh_}")
                          for lh_ in range(HPC)]
                    pts = []

                    def pv(kc):
                        pt, c0 = pts[kc]
                        gc = (Tb // 128) * b + kc
                        for lh in range(HPC):
                            nc.tensor.matmul(
                                po[lh][0:65, c0:512],
                                v_sb[:, 130 * gc + 65 * lh:
                                     130 * gc + 65 * lh + 65],
                                pt[:, 512 * lh + c0: 512 * lh + 512],
                                start=(kc == 0), stop=(kc == kmax - 1),
                                skip_group_check=True)

                    emitted = 0
                    for kc in range(kmax):
                        k0 = Tb * b + P * kc
                        j = kc - 4 * qw
                        c0 = max(0, j) * P
                        ps = psum.tile([P, 1024], F32, tag="pair", bufs=2)
                        pt = ptiles.tile([P, 1024], BF16, tag="pT")
                        pts.append((pt, c0))
                        for lh in range(HPC):
                            hs = 64 * lh
                            nc.tensor.matmul(
                                ps[:, 512 * lh + c0: 512 * lh + 512],
                                kT_sb[hs: hs + 64, k0: k0 + P],
                                qT_sb[hs: hs + 64, q0 + c0: q0 + 512],
                                start=True, stop=True)
                        # one exp for both heads; rank-3 AP skips the dead
                        # prefix of diagonal chunks
                        src = ps[:].rearrange("p (h x) -> p h x", h=2)
                        dst = pt[:].rearrange("p (h x) -> p h x", h=2)
                        nc.scalar.activation(
                            dst[:, :, c0:512], src[:, :, c0:512],
                            mybir.ActivationFunctionType.Exp,
                            scale=float(SCALE))
                        if j >= 0:
                            nc.vector.tensor_mul(
                                dst[:, :, c0: c0 + P],
                                dst[:, :, c0: c0 + P],
                                mk_sb[:].rearrange("p (h x) -> p h x", h=2))
                        # weave filler, then the lag-1 PV
                        target = ((kc + 1) * len(units)) // kmax
                        while emitted < target:
                            units[emitted]()
                            emitted += 1
                        if kc >= 1:
                            pv(kc - 1)
                    pv(kmax - 1)

                    for lh in range(HPC):
                        oa = small.tile([65, 512], BF16, tag="oa")
                        nc.vector.tensor_copy(oa[:], po[lh][0:65, :])
                        off = (q0 % TOKS) % HTOK
                        nc.sync.dma_start(
                            a2a_in[ph][r, 64 * lh: 64 * lh + 64, off: off + 512],
                            oa[0:64, :])
                        nc.sync.dma_start(
                            a2a_in[ph][r, P + lh, off: off + 512],
                            oa[64:65, :])

                    if NPH == 2 and k == B * (NQW - 2) + B - 1:
                        fire_a2a(0)

                # final phase: collective + projection
                fire_a2a(NPH - 1)
                ga1 = proj_norm(NPH - 1)
                for tc2 in range(HTOK // P):
                    for nw in range(NNW):
                        proj_group(NPH - 1, ga1, tc2, nw)

    nc.finalize()
    return nc


def _host_inputs(x, w_attn, w_proj, Tb=T):
    import ml_dtypes
    bf16 = ml_dtypes.bfloat16
    BT = B * Tb
    xT = np.ascontiguousarray(x.reshape(BT, C).T).astype(bf16)
    wproj_bf = np.ascontiguousarray(w_proj).astype(bf16)
    rr = np.arange(P)[:, None]
    cc = np.arange(P)[None, :]
    tri = (rr <= cc).astype(bf16)
    masks = np.concatenate([tri, tri], axis=1)   # [128, 256]
    ident = np.eye(P).astype(bf16)
    ii = np.arange(16)[:, None, None]
    jj = np.arange(8)[None, :, None]
    pp = np.arange(P)[None, None, :]
    sel = (ii == 8 * (pp // 64) + jj).astype(bf16)  # [16, 8, 128]
    in_maps = []
    for g in range(NCORES):
        in_maps.append({
            "xT": xT,
            "wq": np.ascontiguousarray(w_attn[:, DH * g: DH * (g + 1)]).astype(bf16),
            "wk": np.ascontiguousarray(w_attn[:, C + DH * g: C + DH * (g + 1)]).astype(bf16),
            "wv": np.ascontiguousarray(w_attn[:, 2 * C + DH * g: 2 * C + DH * (g + 1)]).astype(bf16),
            "wproj": wproj_bf,
            "masks": masks,
            "sel": sel,
            "ident": ident,
        })
    return in_maps


_NC_CACHE = {}


def kernel(x, w_attn, w_proj):
    x = np.asarray(x)
    w_attn = np.asarray(w_attn)
    w_proj = np.asarray(w_proj)
    if T not in _NC_CACHE:
        _NC_CACHE[T] = build_nc(T)
    nc = _NC_CACHE[T]
    in_maps = _host_inputs(x, w_attn, w_proj, T)
    res = run_bass_kernel_spmd(nc, in_maps, core_ids=list(range(NCORES)))
    y = np.concatenate([res.results[g]["y"] for g in range(NCORES)], axis=0)
    return y.reshape(B, T, C).astype(np.float32)


# revision 37
# speedup vs baseline: 1.9206x; 1.1188x over previous
"""Distributed causal self-attention for 8 TRN2 NeuronCores (v2).

Sharding: tensor-parallel over heads (2 heads/core, all batches); an
AllToAll (split in 2 phases) redistributes the attention output from
head-sharded to token-sharded for the output projection.

v2 structure (per core, all matmuls bf16, fp32 PSUM):
  - qw-major rounds interleaving QKV projection (stage A) with attention
    (stage B): round r computes scores/exp/PV for query window r of all
    batches while the PE also runs stage-A matmuls for window r+1, so the
    ACT engine's exp stream hides under PE work.
  - scores for the two heads are emitted back-to-back as 64-row PE tiles
    (lhsT/rhs base partitions 0 and 64) -> they run concurrently in the
    PE array (row tiling).
  - per key-chunk PSUM tile [128, 1024]: cols 0:512 head0, 512:1024
    head1; ONE exp instruction per chunk with a rank-3 AP that skips the
    causally-dead prefix of diagonal chunks.  Causal triangle masked by a
    bf16 0/1 multiply on the [128,128] diagonal tiles only.
  - PV accumulates O^T (rows 0:64) and the softmax denominator (row 64,
    via a ones column baked into v_sb) in PSUM; the UNNORMALIZED output
    plus denominator rows travel through the AllToAll ([8, 130, 512]
    payload) and normalization happens on the receive side: one fast
    reciprocal + gpsimd partition-broadcasts + one big DVE multiply per
    phase.
"""

import numpy as np

import concourse.bass as bass
import concourse.bacc as bacc
import concourse.mybir as mybir
import concourse.tile as tile
from concourse.bass_utils import run_bass_kernel_spmd
from concourse.dve_ops import RECIPROCAL_APPROX_FAST, RECIP_APPROX_FAST_CONSTS

B, T, C = 4, 2048, 1024
H, D = 16, 64
NCORES = 8
HPC = H // NCORES        # heads per core
DH = HPC * D             # 128 qkv cols per core
P = 128
F32 = mybir.dt.float32
BF16 = mybir.dt.bfloat16
SCALE = 1.0 / np.sqrt(D)


def build_nc(Tb=T, reps=1, debug=False, do_norm=True):
    BT = B * Tb              # total tokens
    NTW = BT // 512          # 512-token windows (stage A units)
    NQW = Tb // 512          # query windows per batch
    NCH = BT // 128          # 128-token chunks total
    TOKS = BT // NCORES      # tokens per core in the proj stage
    NNW = C // 512           # output column windows
    NPH = 2 if Tb >= 2048 else 1
    HTOK = TOKS // NPH       # tokens per proj phase

    nc = bacc.Bacc(None, target_bir_lowering=False)

    xT_ext = nc.declare_dram_parameter("xT", [C, BT], BF16, isOutput=False)
    wq_ext = nc.declare_dram_parameter("wq", [C, DH], BF16, isOutput=False)
    wk_ext = nc.declare_dram_parameter("wk", [C, DH], BF16, isOutput=False)
    wv_ext = nc.declare_dram_parameter("wv", [C, DH], BF16, isOutput=False)
    wp_ext = nc.declare_dram_parameter("wproj", [C, C], BF16, isOutput=False)
    mk_ext = nc.declare_dram_parameter("masks", [P, 256], BF16, isOutput=False)
    sel_ext = nc.declare_dram_parameter("sel", [8, 16, P], BF16, isOutput=False)
    id_ext = nc.declare_dram_parameter("ident", [P, P], BF16, isOutput=False)
    y_ext = nc.declare_dram_parameter("y", [TOKS, C], F32, isOutput=True)
    dbg_ext = (nc.declare_dram_parameter("dbg", [NPH, 2, 16, HTOK], F32,
                                         isOutput=True) if debug else None)
    dbg2_ext = (nc.declare_dram_parameter("dbg2", [3, P, 1024], F32,
                                          isOutput=True) if debug else None)
    dbg3_ext = (nc.declare_dram_parameter("dbg3", [2, 130, 512], BF16,
                                          isOutput=True) if debug else None)
    dbg4_ext = (nc.declare_dram_parameter("dbg4", [P, 8, HTOK], BF16,
                                          isOutput=True) if debug else None)

    xT_v = xT_ext.rearrange("(c p) t -> p c t", p=P)     # [128, 8, BT]
    wq_v = wq_ext.rearrange("(c p) m -> p c m", p=P)
    wk_v = wk_ext.rearrange("(c p) m -> p c m", p=P)
    wv_v = wv_ext.rearrange("(c p) m -> p c m", p=P)
    wp_v = wp_ext.rearrange("(c p) m -> p c m", p=P)     # [128, 8, 1024]

    with tile.TileContext(nc, num_cores=NCORES) as tc:
        with (
            tc.tile_pool(name="consts", bufs=1) as consts,
            tc.tile_pool(name="acts", bufs=1) as acts,
            tc.tile_pool(name="xin", bufs=3) as xin,
            tc.tile_pool(name="small", bufs=4) as small,
            tc.tile_pool(name="ptiles", bufs=6) as ptiles,
            tc.tile_pool(name="psum", bufs=1, space="PSUM") as psum,
            tc.tile_pool(name="dram", bufs=1, space="DRAM") as dram,
        ):
            # ---- constants ----
            wq_sb = consts.tile([P, 8, DH], BF16)
            wk_sb = consts.tile([P, 8, DH], BF16)
            wv_sb = consts.tile([P, 8, DH], BF16)
            wp_sb = consts.tile([P, 8, C], BF16)
            mk_sb = consts.tile([P, 256], BF16)   # [tri | tri]
            id_sb = consts.tile([P, P], BF16)
            sel_sb = consts.tile([8, 16, P], BF16)
            nc.gpsimd.dma_start(sel_sb[:], sel_ext[:])
            nc.gpsimd.dma_start(wq_sb[:], wq_v[:])
            nc.gpsimd.dma_start(wk_sb[:], wk_v[:])
            nc.gpsimd.dma_start(wv_sb[:], wv_v[:])
            nc.gpsimd.dma_start(wp_sb[:], wp_v[:])
            nc.gpsimd.dma_start(mk_sb[:], mk_ext[:])
            nc.gpsimd.dma_start(id_sb[:], id_ext[:])

            # ---- persistent activations ----
            qT_sb = acts.tile([P, BT], BF16)
            kT_sb = acts.tile([P, BT], BF16)
            v_sb = acts.tile([P, 130 * NCH], BF16)
            nc.vector.memset(v_sb[:], 1.0)  # bakes in the ones columns

            a2a_in = [dram.tile([NCORES, 130, HTOK], BF16, name=f"a2ain{p}",
                                tag=f"a2ain{p}") for p in range(NPH)]
            a2a_out = [dram.tile([NCORES, 130, HTOK], BF16, name=f"a2aout{p}",
                                 tag=f"a2aout{p}") for p in range(NPH)]

            def fetch_x(tw):
                xw = xin.tile([P, 8, 512], BF16, tag="xw")
                nc.sync.dma_start(xw[:], xT_v[:, :, 512 * tw: 512 * (tw + 1)])
                return xw

            def stage_a_units(tw, xw):
                """QKV projection for one 512-token window, split into 4
                filler units (~1-2us of PE work each); xw was prefetched."""
                st = {}

                def u1():
                    pq = psum.tile([P, 512], F32, tag="stA", bufs=2)
                    for cc in range(8):
                        nc.tensor.matmul(pq[:], wq_sb[:, cc, :], xw[:, cc, :],
                                         start=(cc == 0), stop=(cc == 7))
                    nc.vector.tensor_copy(qT_sb[:, 512 * tw: 512 * (tw + 1)], pq[:])

                def u2():
                    pk = psum.tile([P, 512], F32, tag="stA", bufs=2)
                    for cc in range(8):
                        nc.tensor.matmul(pk[:], wk_sb[:, cc, :], xw[:, cc, :],
                                         start=(cc == 0), stop=(cc == 7))
                    nc.vector.tensor_copy(kT_sb[:, 512 * tw: 512 * (tw + 1)], pk[:])

                def u3():
                    pvT = psum.tile([P, 512], F32, tag="stA", bufs=2)
                    for cc in range(8):
                        nc.tensor.matmul(pvT[:], wv_sb[:, cc, :], xw[:, cc, :],
                                         start=(cc == 0), stop=(cc == 7))
                    vT = small.tile([P, 512], BF16, tag="vT")
                    nc.vector.tensor_copy(vT[:], pvT[:])
                    st["vT"] = vT

                def u4():
                    vT = st["vT"]
                    for j in range(4):
                        pv = psum.tile([P, P], BF16, tag="stA", bufs=2)
                        nc.tensor.transpose(pv[:], vT[:, P * j: P * (j + 1)],
                                            id_sb[:])
                        gc = 4 * tw + j
                        dst = v_sb[:, 130 * gc: 130 * gc + 130].rearrange(
                            "p (h d) -> p h d", h=2, d=65)[:, :, 0:64]
                        src = pv[:, :].rearrange("p (h d) -> p h d", h=2)
                        nc.vector.tensor_copy(dst, src)

                return [u1, u2, u3, u4]

            def proj_norm(phase):
                """Receive side of one AllToAll phase: load + normalize.
                Returns the normalized [P, 8, HTOK] activation tile."""
                ga = acts.tile([P, 8, HTOK], BF16, tag=f"ga{phase}")
                nc.sync.dma_start(
                    ga[:], a2a_out[phase][:, 0:P, :].rearrange("j p t -> p j t"))
                dn = small.tile([8, 2, HTOK], BF16, tag=f"dn{phase}", bufs=1)
                nc.sync.dma_start(dn[:], a2a_out[phase][:, P:P + 2, :])
                dnf32 = small.tile([8, 2, HTOK], F32, tag=f"dnf32{phase}", bufs=1)
                nc.vector.tensor_copy(dnf32[:], dn[:])
                rf32 = small.tile([8, 2, HTOK], F32, tag=f"rf32{phase}", bufs=1)
                cst = RECIP_APPROX_FAST_CONSTS
                nc.vector._custom_dve(RECIPROCAL_APPROX_FAST, out=rf32[:],
                                      in0=dnf32[:], s0=cst["s0"], s1=cst["s1"],
                                      imm2=cst["imm2"])
                rf = small.tile([8, 2, HTOK], BF16, tag=f"rf{phase}", bufs=1)
                nc.vector.tensor_copy(rf[:], rf32[:])
                if debug:
                    nc.sync.dma_start(
                        dbg_ext[phase, 0].rearrange("(j h) t -> j h t", h=2),
                        dnf32[:])
                if do_norm:
                    # broadcast 1/den along partitions via K=8 select
                    # matmuls, then scale ga straight from PSUM
                    for j in range(NCORES):
                        pb = psum.tile([P, HTOK], F32, tag="stA", bufs=2)
                        for h in range(HPC):
                            nc.tensor.matmul(pb[:], sel_sb[:, 2 * j + h, :],
                                             rf[:, h, :],
                                             start=(h == 0), stop=(h == 1))
                        nc.vector.tensor_mul(ga[:, j, :], ga[:, j, :], pb[:])
                return ga

            def proj_group(phase, ga, tc2, nw):
                """One [128 tok, 512 col] block of the output projection."""
                py = psum.tile([P, 512], F32, tag="stA", bufs=2)
                for cc in range(8):
                    nc.tensor.matmul(
                        py[:], ga[:, cc, P * tc2: P * (tc2 + 1)],
                        wp_sb[:, cc, 512 * nw: 512 * (nw + 1)],
                        start=(cc == 0), stop=(cc == 7))
                ys = small.tile([P, 512], F32, tag="ys")
                nc.vector.tensor_copy(ys[:], py[:])
                nc.sync.dma_start(
                    y_ext[HTOK * phase + P * tc2: HTOK * phase + P * (tc2 + 1),
                          512 * nw: 512 * (nw + 1)],
                    ys[:])

            def fire_a2a(phase):
                nc.gpsimd.collective_compute(
                    "AllToAll", mybir.AluOpType.bypass,
                    replica_groups=[list(range(NCORES))],
                    ins=[a2a_in[phase].opt()], outs=[a2a_out[phase].opt()])

            SEQ = [(qw, b) for qw in range(NQW) for b in range(B)]

            for rep in range(reps):
                ga0 = None
                # prologue: stage A for the first window, prefetch the next
                xw_pend = {}
                qw2, b2 = SEQ[0]
                xw0 = fetch_x(4 * b2 + qw2)
                for u in stage_a_units(4 * b2 + qw2, xw0):
                    u()
                if len(SEQ) > 1:
                    qw2, b2 = SEQ[1]
                    xw_pend[1] = fetch_x(4 * b2 + qw2)

                for k, (qw, b) in enumerate(SEQ):
                    kmax = 4 * qw + 4
                    q0 = Tb * b + 512 * qw
                    r = q0 // TOKS
                    ph = (q0 % TOKS) // HTOK

                    # prefetch x for the window woven NEXT iteration
                    if k + 2 < len(SEQ):
                        qw2, b2 = SEQ[k + 2]
                        xw_pend[k + 2] = fetch_x(4 * b2 + qw2)
                    # ---- filler units for this iteration ----
                    units = []
                    if k + 1 < len(SEQ):
                        qw2, b2 = SEQ[k + 1]
                        units += stage_a_units(4 * b2 + qw2, xw_pend.pop(k + 1))
                    if NPH == 2 and qw == NQW - 1 and b >= B - 2:
                        # C(phase 0) woven into the last two iterations
                        if b == B - 2:
                            def mknorm():
                                nonlocal ga0
                                ga0 = proj_norm(0)
                            units.append(mknorm)
                            units += [
                                (lambda t=t, n=n: proj_group(0, ga0, t, n))
                                for t in range(2) for n in range(NNW)]
                        else:
                            units += [
                                (lambda t=t, n=n: proj_group(0, ga0, t, n))
                                for t in range(2, 4) for n in range(NNW)]

                    po = [psum.tile([P, 512], F32, tag="po", bufs=2,
                                    name=f"po{k}_{lh_}")
                          for lh_ in range(HPC)]
                    pts = []

                    def pv(kc):
                        pt, c0 = pts[kc]
                        gc = (Tb // 128) * b + kc
                        for lh in range(HPC):
                            nc.tensor.matmul(
                                po[lh][0:65, c0:512],
                                v_sb[:, 130 * gc + 65 * lh:
                                     130 * gc + 65 * lh + 65],
                                pt[:, 512 * lh + c0: 512 * lh + 512],
                                start=(kc == 0), stop=(kc == kmax - 1),
                                skip_group_check=True)

                    emitted = 0
                    for kc in range(kmax):
                        k0 = Tb * b + P * kc
                        j = kc - 4 * qw
                        c0 = max(0, j) * P
                        ps = psum.tile([P, 1024], F32, tag="pair", bufs=2)
                        pt = ptiles.tile([P, 1024], BF16, tag="pT")
                        pts.append((pt, c0))
                        for lh in range(HPC):
                            hs = 64 * lh
                            nc.tensor.matmul(
                                ps[:, 512 * lh + c0: 512 * lh + 512],
                                kT_sb[hs: hs + 64, k0: k0 + P],
                                qT_sb[hs: hs + 64, q0 + c0: q0 + 512],
                                start=True, stop=True)
                        # one exp for both heads; rank-3 AP skips the dead
                        # prefix of diagonal chunks
                        src = ps[:].rearrange("p (h x) -> p h x", h=2)
                        dst = pt[:].rearrange("p (h x) -> p h x", h=2)
                        nc.scalar.activation(
                            dst[:, :, c0:512], src[:, :, c0:512],
                            mybir.ActivationFunctionType.Exp,
                            scale=float(SCALE))
                        if j >= 0:
                            nc.vector.tensor_mul(
                                dst[:, :, c0: c0 + P],
                                dst[:, :, c0: c0 + P],
                                mk_sb[:].rearrange("p (h x) -> p h x", h=2))
                        # weave filler, then the lag-1 PV
                        target = ((kc + 1) * len(units)) // kmax
                        while emitted < target:
                            units[emitted]()
                            emitted += 1
                        if kc >= 1:
                            pv(kc - 1)
                    pv(kmax - 1)

                    for lh in range(HPC):
                        oa = small.tile([65, 512], BF16, tag="oa")
                        nc.vector.tensor_copy(oa[:], po[lh][0:65, :])
                        off = (q0 % TOKS) % HTOK
                        nc.sync.dma_start(
                            a2a_in[ph][r, 64 * lh: 64 * lh + 64, off: off + 512],
                            oa[0:64, :])
                        nc.sync.dma_start(
                            a2a_in[ph][r, P + lh, off: off + 512],
                            oa[64:65, :])

                    if NPH == 2 and k == B * (NQW - 2) + B - 1:
                        fire_a2a(0)

                # final phase: collective + projection
                fire_a2a(NPH - 1)
                ga1 = proj_norm(NPH - 1)
                for tc2 in range(HTOK // P):
                    for nw in range(NNW):
                        proj_group(NPH - 1, ga1, tc2, nw)

    nc.finalize()
    return nc


def _host_inputs(x, w_attn, w_proj, Tb=T):
    import ml_dtypes
    bf16 = ml_dtypes.bfloat16
    BT = B * Tb
    xT = np.ascontiguousarray(x.reshape(BT, C).T).astype(bf16)
    wproj_bf = np.ascontiguousarray(w_proj).astype(bf16)
    rr = np.arange(P)[:, None]
    cc = np.arange(P)[None, :]
    tri = (rr <= cc).astype(bf16)
    masks = np.concatenate([tri, tri], axis=1)   # [128, 256]
    ident = np.eye(P).astype(bf16)
    ii = np.arange(8)[:, None, None]
    ss = np.arange(16)[None, :, None]     # slot = 2j + h
    pp = np.arange(P)[None, None, :]
    sel = ((ii == ss // 2) & (pp // 64 == ss % 2)).astype(bf16)  # [8, 16, 128]
    in_maps = []
    for g in range(NCORES):
        in_maps.append({
            "xT": xT,
            "wq": np.ascontiguousarray(w_attn[:, DH * g: DH * (g + 1)]).astype(bf16),
            "wk": np.ascontiguousarray(w_attn[:, C + DH * g: C + DH * (g + 1)]).astype(bf16),
            "wv": np.ascontiguousarray(w_attn[:, 2 * C + DH * g: 2 * C + DH * (g + 1)]).astype(bf16),
            "wproj": wproj_bf,
            "masks": masks,
            "sel": sel,
            "ident": ident,
        })
    return in_maps


_NC_CACHE = {}


def kernel(x, w_attn, w_proj):
    x = np.asarray(x)
    w_attn = np.asarray(w_attn)
    w_proj = np.asarray(w_proj)
    if T not in _NC_CACHE:
        _NC_CACHE[T] = build_nc(T)
    nc = _NC_CACHE[T]
    in_maps = _host_inputs(x, w_attn, w_proj, T)
    res = run_bass_kernel_spmd(nc, in_maps, core_ids=list(range(NCORES)))
    y = np.concatenate([res.results[g]["y"] for g in range(NCORES)], axis=0)
    return y.reshape(B, T, C).astype(np.float32)
